# revision 24
# baseline (speedup 1.0000x reference)
"""Trainium2 Bass kernel for an MoE layer (dense top-2 routing, E=8, D=1024).

Strategy: data-parallel over tokens across 8 NeuronCores (1024 tokens/core).
Each core computes all 8 experts densely on its token slice in a fully
transposed layout (activations stored [feature, token]) so that all matmuls
use natural-layout weights as the stationary operand and fp32r (full fp32
data, fast PE streaming mode) throughput.  The router / softmax / top-2
gating is computed on-device in plain fp32; the gate is folded into the
hidden activations before the second matmul, and expert outputs accumulate
into an SBUF accumulator.  Outputs per core: outT [D, 1024] (combined,
transposed) and logits [1024, E].  The host only transposes/concatenates.
"""

import threading

import numpy as np

D = 1024
E = 8
NCORES = 8
B, S = 4, 2048
T = B * S            # 8192 tokens
TOK = T // NCORES    # 1024 tokens per core
P = 128
KT = D // P          # 8 contraction tiles
MT = D // P          # 8 hidden tiles
NT = D // P          # 8 output tiles
TT = TOK // P        # 8 token tiles of 128
TH = 2               # token halves for the big matmuls
THW = TOK // TH      # 512 (max fp32 moving free dim)

_cache = {}
_lock = threading.Lock()


def _patch_tile_drain():
    """This container's walrus build rejects >1 sem wait on an InstDrain;
    chunk the TileContext tail-drain waits across multiple drains."""
    import concourse.mybir as mybir
    import concourse.tile as tile
    from concourse.vector_clock import ScopedClock

    if getattr(tile.TileContext, "_moe_drain_patched", False):
        return
    _MAXW = 1

    def _patched(self, tick_clock, wait_clock):
        nc = self.nc
        drain_inst = nc.sync.drain()
        wait_clock.add_sem_waits(
            drain_inst.ins, ScopedClock({None: tick_clock.global_clock})
        )
        si = drain_inst.ins.sync_info
        if si is not None and si.on_wait and len(si.on_wait) > _MAXW:
            waits = list(si.on_wait)
            upd = list(si.on_update) if si.on_update else []
            drain_inst.ins.sync_info = mybir.SyncInfo(
                on_wait=waits[:_MAXW], on_update=upd
            )
            rest = waits[_MAXW:]
            while rest:
                extra = nc.sync.drain()
                extra.ins.sync_info = mybir.SyncInfo(on_wait=rest[:_MAXW], on_update=[])
                rest = rest[_MAXW:]
        nc.all_engine_barrier()
        popped = nc._tile_sem_poison_stack.pop()
        assert popped is self._sem_poison
        nc.clear_and_free_semaphores(list(self.sems.allocated().values()))
        nc.all_engine_barrier()

    tile.TileContext._drain_and_barrier = _patched
    tile.TileContext._moe_drain_patched = True


def _moe_body(tc, aps, include_b2, passes=1):
    import concourse.mybir as mybir
    from concourse.alu_op_type import AluOpType as Alu
    from concourse.masks import make_identity

    nc = tc.nc
    dt = mybir.dt
    f32, f32r = dt.float32, dt.float32r
    F = mybir.ActivationFunctionType
    AX = mybir.AxisListType.X

    xT, wr, w1, b1, w2, b2, outT, logits_o = aps

    with (
        tc.tile_pool(name="xp", bufs=1) as xp,
        tc.tile_pool(name="constp", bufs=1) as constp,
        tc.tile_pool(name="w1p", bufs=1) as w1p,
        tc.tile_pool(name="w2p", bufs=1) as w2p,
        tc.tile_pool(name="hp", bufs=1) as hp,
        tc.tile_pool(name="accp", bufs=1) as accp,
        tc.tile_pool(name="gp", bufs=2) as gp,
        tc.tile_pool(name="smallp", bufs=2) as smallp,
        tc.tile_pool(name="psA", bufs=3, space="PSUM") as psA,
        tc.tile_pool(name="psB", bufs=3, space="PSUM") as psB,
        tc.tile_pool(name="psS", bufs=2, space="PSUM") as psS,
    ):
        # ---- resident inputs -------------------------------------------------
        # PE-facing tiles are f32r (fast PE mode, full fp32 bytes); the router
        # reads the same bytes through an f32 bitcast for exact selection.
        x_sb = []
        for k in range(KT):
            xk = xp.tile([P, TOK], f32r, tag=f"x{k}", name=f"x_sb{k}")
            nc.sync.dma_start(xk[:], xT[k * P:(k + 1) * P, :].bitcast(f32r))
            x_sb.append(xk)

        wr_sb = []
        for k in range(KT):
            wrk = constp.tile([P, E], f32, tag=f"wr{k}", name=f"wr_sb{k}")
            nc.sync.dma_start(wrk[:], wr[k * P:(k + 1) * P, :])
            wr_sb.append(wrk)

        # b1/b2 as [128, E*MT]: column e*MT+m holds b[e, m*128:(m+1)*128]
        b1_sb = constp.tile([P, E * MT], f32, tag="b1", name="b1_sb")
        b2_sb = constp.tile([P, E * NT], f32, tag="b2", name="b2_sb")
        for e in range(E):
            b1e = b1[e, :].rearrange("(m p) -> p m", p=P)
            nc.sync.dma_start(b1_sb[:, e * MT:(e + 1) * MT], b1e)
            b2e = b2[e, :].rearrange("(m p) -> p m", p=P)
            nc.sync.dma_start(b2_sb[:, e * NT:(e + 1) * NT], b2e)

        ident = constp.tile([P, P], f32, tag="ident", name="ident")
        make_identity(nc, ident)
        ones_row = constp.tile([1, P], f32, tag="ones_row", name="ones_row")
        nc.vector.memset(ones_row[:], 1.0)

        # ---- router: logits[t, e] = x[t, :] @ wr -----------------------------
        # The f32r DMA rounds values (~12-bit mantissa), which is fine for the
        # expert matmuls but corrupts top-2 selection.  Re-DMA x as exact f32
        # in transient [128,128] tiles for the router matmul.
        gate_sb = []  # per token tile: [128, E] fp32 gate weights
        for tt in range(TT):
            xr_sb = []
            for k in range(KT):
                xr = smallp.tile([P, P], f32, tag=f"xr{k}", bufs=2,
                                 name=f"xr{tt}_{k}")
                nc.sync.dma_start(
                    xr[:], xT[k * P:(k + 1) * P, tt * P:(tt + 1) * P]
                )
                xr_sb.append(xr)
            ps_l = psS.tile([P, P], f32, tag="s", name=f"ps_l{tt}")
            for k in range(KT):
                nc.tensor.matmul(
                    ps_l[:, :E],
                    xr_sb[k][:],
                    wr_sb[k][:],
                    start=(k == 0),
                    stop=(k == KT - 1),
                )
            lg = smallp.tile([P, E], f32, tag="lg", bufs=3, name=f"lg{tt}")
            nc.vector.tensor_copy(lg[:], ps_l[:, :E])
            nc.sync.dma_start(logits_o[tt * P:(tt + 1) * P, :], lg[:])

            # top-2 gating: gate = softmax(logits) masked to the top-2 entries
            mx = smallp.tile([P, 8], f32, tag="mx", bufs=3, name=f"mx{tt}")
            nc.vector.max(out=mx[:], in_=lg[:])
            negm1 = smallp.tile([P, 1], f32, tag="negm1", bufs=3, name=f"negm1{tt}")
            nc.vector.tensor_scalar_mul(negm1[:], mx[:, 0:1], -1.0)
            expv = smallp.tile([P, E], f32, tag="expv", bufs=3, name=f"expv{tt}")
            nc.scalar.activation(expv[:], lg[:], F.Exp, bias=negm1[:], scale=1.0)
            den = smallp.tile([P, 1], f32, tag="den", bufs=3, name=f"den{tt}")
            nc.vector.reduce_sum(den[:], expv[:], axis=AX)
            rec = smallp.tile([P, 1], f32, tag="rec", bufs=3, name=f"rec{tt}")
            nc.vector.reciprocal(rec[:], den[:])
            mask = smallp.tile([P, E], f32, tag="mask", bufs=3, name=f"mask{tt}")
            nc.vector.tensor_tensor(
                mask[:], lg[:], mx[:, 1:2].to_broadcast([P, E]), Alu.is_ge
            )
            gt = smallp.tile([P, E], f32, tag=f"gt{tt}", name=f"gt{tt}")
            nc.vector.tensor_tensor(gt[:], expv[:], mask[:], Alu.mult)
            nc.vector.tensor_scalar_mul(gt[:], gt[:], rec[:])
            gate_sb.append(gt)

        # ---- accumulator -----------------------------------------------------
        acc = []
        for n in range(NT):
            an = accp.tile([P, TOK], f32, tag=f"a{n}", name=f"acc{n}")
            acc.append(an)

        # ---- expert loop -----------------------------------------------------
        for e0 in range(passes * E):
            e = e0 % E
            w1_sb = []
            for k in range(KT):
                w1k = w1p.tile([P, D], f32r, tag=f"w1k{k}", name=f"w1_{e}_{k}")
                nc.sync.dma_start(w1k[:], w1[e, k * P:(k + 1) * P, :].bitcast(f32r))
                w1_sb.append(w1k)
            w2_sb = []
            for k in range(KT):
                w2k = w2p.tile([P, D], f32r, tag=f"w2k{k}", name=f"w2_{e}_{k}")
                nc.sync.dma_start(w2k[:], w2[e, k * P:(k + 1) * P, :].bitcast(f32r))
                w2_sb.append(w2k)

            # gate row for this expert → partition 0, then broadcast to 128
            stag = gp.tile([1, TOK], f32, tag="stag", name=f"stag{e}")
            for tt in range(TT):
                ps_t = psS.tile([1, P], f32, tag="s", name=f"ps_t{e}_{tt}")
                nc.tensor.transpose(ps_t[:], gate_sb[tt][:, e:e + 1], ident[:])
                nc.vector.tensor_copy(stag[:, tt * P:(tt + 1) * P], ps_t[:])
            G = gp.tile([P, TOK], f32, tag="G", name=f"G{e}")
            for th in range(TH):
                ts = slice(th * THW, (th + 1) * THW)
                psG = psS.tile([P, THW], f32, tag="s", name=f"psG{e}{th}")
                nc.tensor.matmul(psG[:], ones_row[:], stag[:, ts],
                                 start=True, stop=True)
                nc.vector.tensor_copy(G[:, ts], psG[:])

            for th in range(TH):
                ts = slice(th * THW, (th + 1) * THW)
                # h^T[m-tile] = gelu(w1^T x^T + b1), then scaled by the gate
                hs = []
                for m in range(MT):
                    ps_h = psA.tile([P, THW], f32, tag="ph", name=f"psh{e}{th}{m}")
                    for k in range(KT):
                        nc.tensor.matmul(
                            ps_h[:],
                            w1_sb[k][:, m * P:(m + 1) * P],
                            x_sb[k][:, ts],
                            start=(k == 0),
                            stop=(k == KT - 1),
                        )
                    hm = hp.tile([P, THW], f32r, tag=f"h{m}", name=f"h{e}{th}{m}")
                    nc.scalar.activation(
                        hm[:], ps_h[:], F.Gelu_apprx_tanh,
                        bias=b1_sb[:, e * MT + m:e * MT + m + 1], scale=1.0,
                    )
                    hs.append(hm)

                # out^T[n-tile] += g * (w2^T h^T)   (+ g ⊗ b2)
                for n in range(NT):
                    ps_o = psB.tile([P, THW], f32, tag="po", name=f"pso{e}{th}{n}")
                    for k in range(KT):
                        nc.tensor.matmul(
                            ps_o[:],
                            w2_sb[k][:, n * P:(n + 1) * P],
                            hs[k][:],
                            start=(k == 0),
                            stop=(k == KT - 1),
                        )
                    if e == 0:
                        nc.vector.tensor_tensor(
                            acc[n][:, ts], ps_o[:], G[:, ts], Alu.mult
                        )
                    else:
                        tmp = smallp.tile([P, THW], f32, tag="tmp", bufs=4,
                                          name=f"tmp{e}{th}{n}")
                        nc.vector.tensor_tensor(tmp[:], ps_o[:], G[:, ts], Alu.mult)
                        nc.vector.tensor_tensor(
                            acc[n][:, ts], acc[n][:, ts], tmp[:], Alu.add
                        )
                    if include_b2:
                        nc.vector.scalar_tensor_tensor(
                            out=acc[n][:, ts],
                            in0=G[:, ts],
                            scalar=b2_sb[:, e * NT + n:e * NT + n + 1],
                            in1=acc[n][:, ts],
                            op0=Alu.mult,
                            op1=Alu.add,
                        )

        for n in range(NT):
            nc.sync.dma_start(outT[n * P:(n + 1) * P, :], acc[n][:])


def _split_excess_waits(nc, limit=1):
    """This walrus build allows at most `limit` sem waits per instruction;
    hoist extra waits onto same-engine nops inserted right before."""
    import concourse.mybir as mybir

    for fn in nc.m.functions:
        for blk in fn.blocks:
            new_insts = []
            for inst in blk.instructions:
                si = inst.sync_info
                waits = list(si.on_wait) if si and si.on_wait else []
                if len(waits) > limit:
                    extra, keep = waits[:-limit], waits[-limit:]
                    for w in extra:
                        nop = mybir.InstNoOp(
                            name=f"I-waitnop-{nc.next_id()}",
                            engine=inst.engine,
                            bass_nofuse=True,
                            sync_info=mybir.SyncInfo(on_wait=[w], on_update=[]),
                        )
                        new_insts.append(nop)
                    inst.sync_info = mybir.SyncInfo(
                        on_wait=keep,
                        on_update=list(si.on_update) if si.on_update else [],
                    )
                new_insts.append(inst)
            blk.instructions = new_insts


def build_nc(include_b2=True, passes=1):
    import concourse.bass as bass
    import concourse.mybir as mybir
    import concourse.tile as tile

    _patch_tile_drain()
    dt = mybir.dt
    f32 = dt.float32

    nc = bass.Bass("TRN2", target_bir_lowering=False, debug=False,
                   num_devices=NCORES)
    xT = nc.dram_tensor("xT", [D, TOK], f32, kind="ExternalInput").ap()
    wr = nc.dram_tensor("wr", [D, E], f32, kind="ExternalInput").ap()
    w1 = nc.dram_tensor("w1", [E, D, D], f32, kind="ExternalInput").ap()
    b1 = nc.dram_tensor("b1", [E, D], f32, kind="ExternalInput").ap()
    w2 = nc.dram_tensor("w2", [E, D, D], f32, kind="ExternalInput").ap()
    b2 = nc.dram_tensor("b2", [E, D], f32, kind="ExternalInput").ap()
    outT = nc.dram_tensor("outT", [D, TOK], f32, kind="ExternalOutput").ap()
    logits_o = nc.dram_tensor("logits", [TOK, E], f32, kind="ExternalOutput").ap()

    with tile.TileContext(nc) as tc:
        _moe_body(tc, (xT, wr, w1, b1, w2, b2, outT, logits_o), include_b2,
                  passes=passes)
    _split_excess_waits(nc)
    return nc


def _get_nc(include_b2, passes=1):
    key = ("nc", include_b2, passes)
    with _lock:
        if key not in _cache:
            _cache[key] = build_nc(include_b2, passes)
        return _cache[key]


def _make_in_maps(x, w_router, w1, b1, w2, b2):
    x_flat = np.ascontiguousarray(np.asarray(x, dtype=np.float32)).reshape(T, D)
    w_router = np.ascontiguousarray(np.asarray(w_router, dtype=np.float32))
    w1 = np.ascontiguousarray(np.asarray(w1, dtype=np.float32))
    b1 = np.ascontiguousarray(np.asarray(b1, dtype=np.float32))
    w2 = np.ascontiguousarray(np.asarray(w2, dtype=np.float32))
    b2 = np.ascontiguousarray(np.asarray(b2, dtype=np.float32))
    in_maps = []
    for c in range(NCORES):
        xTc = np.ascontiguousarray(x_flat[c * TOK:(c + 1) * TOK, :].T)
        in_maps.append({
            "xT": xTc, "wr": w_router, "w1": w1, "b1": b1, "w2": w2, "b2": b2,
        })
    return in_maps


def _assemble(results):
    combined = np.empty((T, D), dtype=np.float32)
    logits = np.empty((T, E), dtype=np.float32)
    for c in range(NCORES):
        combined[c * TOK:(c + 1) * TOK, :] = results[c]["outT"].T
        logits[c * TOK:(c + 1) * TOK, :] = results[c]["logits"]
    return combined.reshape(B, S, D), logits.reshape(B, S, E)


def _get_runner(include_b2, passes=1):
    """Build (once) a reusable jitted shard_map callable for the program."""
    key = ("runner", include_b2, passes)
    with _lock:
        if key in _cache:
            return _cache[key]

    import jax
    from jax.experimental.shard_map import shard_map
    from jax.sharding import Mesh, PartitionSpec

    import concourse.bass2jax as bass2jax
    import concourse.mybir as mybir

    nc = _get_nc(include_b2, passes)
    bass2jax.install_neuronx_cc_hook()

    partition_name = (
        nc.partition_id_tensor.name if nc.partition_id_tensor else None
    )
    in_names, out_names, out_avals, zero_outs = [], [], [], []
    for alloc in nc.m.functions[0].allocations:
        if not isinstance(alloc, mybir.MemoryLocationSet):
            continue
        name = alloc.memorylocations[0].name
        if alloc.kind == "ExternalInput":
            if name != partition_name:
                in_names.append(name)
        elif alloc.kind == "ExternalOutput":
            out_names.append(name)
            shape = tuple(alloc.tensor_shape)
            dtype = mybir.dt.np(alloc.dtype)
            out_avals.append(jax.core.ShapedArray(shape, dtype))
            zero_outs.append(np.zeros((NCORES * shape[0], *shape[1:]), dtype))
    n_params = len(in_names)
    bind_in_names = list(in_names) + list(out_names)
    if partition_name is not None:
        bind_in_names.append(partition_name)

    def _body(*args):
        operands = list(args)
        if partition_name is not None:
            operands.append(bass2jax.partition_id_tensor())
        outs = bass2jax._bass_exec_p.bind(
            *operands,
            out_avals=tuple(out_avals),
            in_names=tuple(bind_in_names),
            out_names=tuple(out_names),
            lowering_input_output_aliases=(),
            sim_require_finite=True,
            sim_require_nnan=True,
            nc=nc,
        )
        return tuple(outs)

    devices = jax.devices()[:NCORES]
    mesh = Mesh(np.asarray(devices), ("core",))
    in_specs = (PartitionSpec("core"),) * (n_params + len(out_names))
    out_specs = (PartitionSpec("core"),) * len(out_names)
    fn = jax.jit(
        shard_map(_body, mesh=mesh, in_specs=in_specs, out_specs=out_specs,
                  check_rep=False),
        keep_unused=True,
    )
    runner = {
        "fn": fn, "mesh": mesh, "in_names": in_names, "out_names": out_names,
        "out_avals": out_avals, "zero_outs": zero_outs,
        "spec": PartitionSpec("core"),
    }
    with _lock:
        _cache[key] = runner
    return runner


def _device_args(runner, in_maps):
    import jax
    from jax.sharding import NamedSharding

    sharding = NamedSharding(runner["mesh"], runner["spec"])
    concat_in = [
        np.concatenate([in_maps[c][name] for c in range(NCORES)], axis=0)
        for name in runner["in_names"]
    ]
    args = [jax.device_put(a, sharding) for a in concat_in]
    args += [jax.device_put(z, sharding) for z in runner["zero_outs"]]
    return args


def _run(runner, args):
    outs = runner["fn"](*args)
    results = [
        {
            name: np.asarray(outs[i]).reshape(
                NCORES, *runner["out_avals"][i].shape
            )[c]
            for i, name in enumerate(runner["out_names"])
        }
        for c in range(NCORES)
    ]
    return results


def kernel(x, w_router, w1, b1, w2, b2):
    include_b2 = bool(np.any(np.asarray(b2)))
    runner = _get_runner(include_b2)
    in_maps = _make_in_maps(x, w_router, w1, b1, w2, b2)
    args = _device_args(runner, in_maps)
    return _assemble(_run(runner, args))


# ---------------------------------------------------------------------------
# development helpers (not used by the grading harness)
# ---------------------------------------------------------------------------

def _numpy_ref(x, w_router, w1, b1, w2, b2):
    x_flat = x.reshape(T, D).astype(np.float64)
    logits = x_flat @ w_router.astype(np.float64)
    lg = logits - logits.max(axis=1, keepdims=True)
    p = np.exp(lg)
    p /= p.sum(axis=1, keepdims=True)
    thresh = np.sort(logits, axis=1)[:, -2:-1]
    gate = p * (logits >= thresh)
    out = np.zeros((T, D))
    for e in range(E):
        h = x_flat @ w1[e].astype(np.float64) + b1[e]
        h = 0.5 * h * (1.0 + np.tanh(np.sqrt(2 / np.pi) * (h + 0.044715 * h ** 3)))
        y = h @ w2[e].astype(np.float64) + b2[e]
        out += gate[:, e:e + 1] * y
    return out.reshape(B, S, D), logits.reshape(B, S, E)


def _patch_sim_gelu():
    """CoreSim has no Gelu implementation; emulate Gelu_apprx_tanh (sim only)."""
    import concourse.bass_interp as bi
    import concourse.mybir as mb

    F = mb.ActivationFunctionType
    if getattr(bi.InstructionExecutor, "_moe_gelu_patched", False):
        return
    orig = bi.InstructionExecutor.visit_InstActivation

    def patched(self, instruction, *, reg_snapshot=None):
        if instruction.func == F.Gelu_apprx_tanh:
            try:
                instruction.func = F.Identity
                res = orig(self, instruction, reg_snapshot=reg_snapshot)
            finally:
                instruction.func = F.Gelu_apprx_tanh
            out_view = self.view_ap(
                instruction.outs[0], bi.Direction.WRITE, instruction,
                reg_snapshot=reg_snapshot,
            )
            h = out_view.astype(np.float32)
            g = 0.5 * h * (1.0 + np.tanh(np.sqrt(2 / np.pi) * (h + 0.044715 * h ** 3)))
            out_view[...] = g.astype(out_view.dtype)
            return res
        return orig(self, instruction, reg_snapshot=reg_snapshot)

    bi.InstructionExecutor.visit_InstActivation = patched
    bi.InstructionExecutor._moe_gelu_patched = True


def _sim_check():
    """Run core 0 in CoreSim and compare against numpy."""
    from concourse.bass_interp import CoreSim

    _patch_sim_gelu()

    rng = np.random.default_rng(0)
    s = 0.02
    x = rng.standard_normal((B, S, D), dtype=np.float32)
    w_router = (rng.standard_normal((D, E)) * s).astype(np.float32)
    w1 = (rng.standard_normal((E, D, D)) * s).astype(np.float32)
    b1 = np.zeros((E, D), np.float32)
    w2 = (rng.standard_normal((E, D, D)) * s).astype(np.float32)
    b2 = np.zeros((E, D), np.float32)

    nc = _get_nc(False)
    in_maps = _make_in_maps(x, w_router, w1, b1, w2, b2)
    sim = CoreSim(nc)
    for name, arr in in_maps[0].items():
        sim.tensor(name)[:] = arr
    sim.simulate()
    outT = np.array(sim.tensor("outT"))
    logits = np.array(sim.tensor("logits"))

    ref_out, ref_logits = _numpy_ref(x, w_router, w1, b1, w2, b2)
    ref_out = ref_out.reshape(T, D)[:TOK]
    ref_logits = ref_logits.reshape(T, E)[:TOK]
    le = np.abs(logits - ref_logits).max()
    oe = np.abs(outT.T - ref_out).max() / np.abs(ref_out).max()
    print("sim logits absmax err:", le)
    print("sim out rel err:", oe)


if __name__ == "__main__":
    import sys
    if "--sim" in sys.argv:
        _sim_check()
